# revision 17
# baseline (speedup 1.0000x reference)
"""Distributed Trainium2 kernel for the image-captioning model
(Linear+BN image embed -> 2-layer LSTM (T=64) -> H->V=32000 projection).

Sharding: the LSTM hidden state is sharded over the 4H gate dimension
(each of the 8 cores owns 128 h-positions of each layer); the per-step
full-h is re-assembled with one AllGather per pipeline stage (layer 1
runs one step behind layer 0, so both layers share a single exchange).
The fc projection is sharded over V (4000 rows/core) and interleaved
into the PE idle gaps of the recurrence. All matmuls run in bf16 with
fp32 PSUM accumulation.
"""
import numpy as np
import ml_dtypes

import concourse.bass as bass
import concourse.bacc as bacc
import concourse.mybir as mybir
from concourse.tile import TileContext
from concourse.tile_rust import add_dep_helper
from concourse.bass_utils import run_bass_kernel_spmd

BF16 = ml_dtypes.bfloat16
F32 = np.float32

V, H, E, B, T = 32000, 1024, 512, 32, 64
NC = 8
HS = H // NC          # 128 h-positions per core
VS = V // NC          # 4000 vocab rows per core
TOK = B * T           # 2048 tokens, col = t*32 + b
EPS = 1e-5
LAG = 4  # layer-1 pipeline lag (Wih1 batched per 4 steps)
# gate slot order in psum: i, f, o, g  (PyTorch weight row blocks i,f,g,o)
GATE_BASE = [0, 1, 3, 2]

_nc_cache = None


# The gathered h lands in SBUF via one contiguous 1KB-per-partition DMA
# (cc_out row 8p+j -> partition p, block j), so the K dim of every
# weight that consumes gathered h is pre-permuted to match: tile j,
# K-partition p draws from h index 8p+j.
_HPERM = (8 * np.arange(128)[None, :] + np.arange(8)[:, None]).reshape(-1)


def _gate_tiles(W, r, nk, interleave=False):
    """Per-core lhsT tile layout for a 4H-row weight: returns
    [128, 4*nk*128] with tile (j, k) at cols (j*nk+k)*128."""
    blocks = np.stack(
        [W[GATE_BASE[j] * H + r * HS : GATE_BASE[j] * H + r * HS + HS, :].T
         for j in range(4)]
    )  # (4, K, 128)
    K = blocks.shape[1]
    assert K == nk * 128
    if interleave:
        assert nk == 8
        blocks = blocks[:, _HPERM, :]
    return np.ascontiguousarray(
        blocks.reshape(4, nk, 128, 128).transpose(2, 0, 1, 3).reshape(128, 4 * nk * 128)
    ).astype(BF16)


def _prep(inputs):
    imgT = np.ascontiguousarray(inputs["image_feat"].T)  # (2048, 32)
    # rhs tiles [128, 16*32]
    imgT_s = np.ascontiguousarray(
        imgT.reshape(16, 128, B).transpose(1, 0, 2).reshape(128, 16 * B)
    ).astype(BF16)
    # lin lhsT tiles [128, 16k*4m*128]
    A = inputs["lin_W"].T  # (2048, 512)
    linWT = np.ascontiguousarray(
        A.reshape(16, 128, 4, 128).transpose(1, 0, 2, 3).reshape(128, 8192)
    ).astype(BF16)

    def col4(x):
        return np.ascontiguousarray(x.reshape(4, 128).T).astype(F32)

    bng = col4(inputs["bn_gamma"])
    bnb = col4(inputs["bn_beta"])

    caps = np.asarray(inputs["captions"])[:, : T - 1]  # (32, 63)
    cap_emb = inputs["emb"][caps]  # (32, 63, 512)
    capT = np.ascontiguousarray(cap_emb.transpose(2, 1, 0).reshape(E, (T - 1) * B)).astype(BF16)

    ident = np.eye(128, dtype=BF16)

    common = {
        "imgT": imgT_s, "linWT": linWT, "bng": bng, "bnb": bnb,
        "capT": capT, "ident": ident,
    }

    b0 = inputs["lstm_bih0"] + inputs["lstm_bhh0"]
    b1 = inputs["lstm_bih1"] + inputs["lstm_bhh1"]
    fcW = inputs["fc_W"]
    fcb_full = inputs["fc_b"]

    in_maps = []
    for r in range(NC):
        m = dict(common)
        m["w0i"] = _gate_tiles(inputs["lstm_Wih0"], r, 4)   # (128, 2048)
        m["w0h"] = _gate_tiles(inputs["lstm_Whh0"], r, 8, True)   # (128, 4096)
        m["w1i"] = _gate_tiles(inputs["lstm_Wih1"], r, 8, True)
        m["w1h"] = _gate_tiles(inputs["lstm_Whh1"], r, 8, True)
        m["b0"] = np.ascontiguousarray(
            np.stack([b0[GATE_BASE[j] * H + r * HS : GATE_BASE[j] * H + r * HS + HS]
                      for j in range(4)], axis=1)).astype(F32)  # (128, 4)
        m["b1"] = np.ascontiguousarray(
            np.stack([b1[GATE_BASE[j] * H + r * HS : GATE_BASE[j] * H + r * HS + HS]
                      for j in range(4)], axis=1)).astype(F32)
        m["b1bc"] = np.ascontiguousarray(np.repeat(m["b1"], B, axis=1)).astype(BF16)
        F = np.zeros((4096, H), np.float32)
        F[:VS] = fcW[r * VS : (r + 1) * VS]
        m["fcw"] = np.ascontiguousarray(
            F.T[_HPERM].reshape(8, 128, 32, 128).transpose(1, 2, 0, 3)
            .reshape(128, 32768)
        ).astype(BF16)
        fb = np.zeros((4096,), np.float32)
        fb[:VS] = fcb_full[r * VS : (r + 1) * VS]
        m["fcb"] = np.ascontiguousarray(fb.reshape(32, 128).T).astype(F32)
        in_maps.append(m)
    return in_maps


def _build():
    global _nc_cache
    if _nc_cache is not None:
        return _nc_cache
    bf = mybir.dt.bfloat16
    f32 = mybir.dt.float32
    nc = bacc.Bacc("TRN2", target_bir_lowering=False, debug=False)

    P = nc.declare_dram_parameter
    d_imgT = P("imgT", [128, 16 * B], bf, isOutput=False)
    d_linWT = P("linWT", [128, 8192], bf, isOutput=False)
    d_bng = P("bng", [128, 4], f32, isOutput=False)
    d_bnb = P("bnb", [128, 4], f32, isOutput=False)
    d_capT = P("capT", [E, (T - 1) * B], bf, isOutput=False)
    d_ident = P("ident", [128, 128], bf, isOutput=False)
    d_w0i = P("w0i", [128, 2048], bf, isOutput=False)
    d_w0h = P("w0h", [128, 4096], bf, isOutput=False)
    d_w1i = P("w1i", [128, 4096], bf, isOutput=False)
    d_w1h = P("w1h", [128, 4096], bf, isOutput=False)
    d_b0 = P("b0", [128, 4], f32, isOutput=False)
    d_b1 = P("b1", [128, 4], f32, isOutput=False)
    d_b1bc = P("b1bc", [128, 128], bf, isOutput=False)
    d_fcw = P("fcw", [128, 32768], bf, isOutput=False)
    d_fcb = P("fcb", [128, 32], f32, isOutput=False)
    d_out = P("out", [VS, TOK], f32, isOutput=True)

    S = nc.alloc_sbuf_tensor
    linWT_s = S("linWT_s", [128, 8192], bf)
    imgT_s = S("imgT_s", [128, 16 * B], bf)
    w0i_s = S("w0i_s", [128, 2048], bf)
    w0h_s = S("w0h_s", [128, 4096], bf)
    w1i_s = S("w1i_s", [128, 4096], bf)
    w1h_s = S("w1h_s", [128, 4096], bf)
    fcw_s = S("fcw_s", [128, 32768], bf)
    xsT_s = S("xsT_s", [128, 8192], bf)
    xp_s = S("xp_s", [128, 8192], bf)
    hs1_s = S("hs1_s", [128, 16384], bf)
    hbuf_s = [S(f"hbuf{i}", [128, NC * 64], bf) for i in range(4)]
    hsend_s = [S(f"hsend{i}", [128, 64], bf) for i in range(2)]
    fst_s = [S(f"fst{i}", [128, 256], f32) for i in range(4)]
    c01_s = S("c01_s", [128, 2 * B], f32)
    sgc_s = [S(f"sgc{i}", [128, 256], f32) for i in range(2)]
    tc01_s = S("tc01_s", [128, 2 * B], f32)
    m1_s = S("m1_s", [128, 2 * B], f32)
    m2_s = S("m2_s", [128, 2 * B], f32)
    b0_s = S("b0_s", [128, 4], f32)
    b1_s = S("b1_s", [128, 4], f32)
    b1bc_s = S("b1bc_s", [128, 128], bf)
    fcb_s = S("fcb_s", [128, 32], f32)
    bng_s = S("bng_s", [128, 4], f32)
    bnb_s = S("bnb_s", [128, 4], f32)
    ident_s = S("ident_s", [128, 128], bf)
    mu_s = S("mu_s", [128, 4], f32)
    e2_s = S("e2_s", [128, 4], f32)
    var_s = S("var_s", [128, 4], f32)
    sc_s = S("sc_s", [128, 4], f32)
    sh_s = S("sh_s", [128, 4], f32)
    tsq_s = S("tsq_s", [128, 128], f32)

    PS = nc.alloc_psum_tensor
    gb = [PS(f"gb{i}", [128, 512], f32) for i in range(2)]
    fbk = [PS(f"fb{i}", [128, 512], f32) for i in range(4)]

    warm_in = nc.dram_tensor("warm_in", [1, 16], f32)
    warm_out = nc.dram_tensor("warm_out", [1, 16], f32, addr_space="Shared")
    cc_in = [nc.dram_tensor(f"cc_in{i}", [128, 64], bf) for i in range(4)]
    cc_out = [nc.dram_tensor(f"cc_out{i}", [NC * 128, 64], bf, addr_space="Shared")
              for i in range(4)]

    ACT = mybir.ActivationFunctionType
    AX = mybir.AxisListType

    with TileContext(nc) as tc:
        dma = nc.sync.dma_start
        # warm up the collective path concurrently with the prologue
        nc.gpsimd.collective_compute(
            "AllReduce",
            mybir.AluOpType.add,
            replica_groups=[list(range(NC))],
            ins=[warm_in[:]],
            outs=[warm_out[:]],
        )
        # --- weight / const loads ---
        dma(out=linWT_s[:], in_=d_linWT[:])
        dma(out=imgT_s[:], in_=d_imgT[:])
        dma(out=w0i_s[:], in_=d_w0i[:])
        dma(out=w0h_s[:], in_=d_w0h[:])
        dma(out=w1i_s[:], in_=d_w1i[:])
        dma(out=w1h_s[:], in_=d_w1h[:])
        dma(out=b0_s[:], in_=d_b0[:])
        dma(out=b1_s[:], in_=d_b1[:])
        dma(out=b1bc_s[:], in_=d_b1bc[:])
        dma(out=fcb_s[:], in_=d_fcb[:])
        dma(out=bng_s[:], in_=d_bng[:])
        dma(out=bnb_s[:], in_=d_bnb[:])
        dma(out=ident_s[:], in_=d_ident[:])
        for k in range(4):
            nc.gpsimd.dma_start(out=xsT_s[:, k * 2048 + B : (k + 1) * 2048],
                                in_=d_capT[k * 128 : (k + 1) * 128, :])
        nc.gpsimd.dma_start(out=fcw_s[:], in_=d_fcw[:])

        # --- image embed: x.T tiles -> gb[0][:, 0:128] ---
        for m in range(4):
            for k in range(16):
                nc.tensor.matmul(
                    gb[0][:, m * B : (m + 1) * B],
                    linWT_s[:, (k * 4 + m) * 128 : (k * 4 + m + 1) * 128],
                    imgT_s[:, k * B : (k + 1) * B],
                    start=(k == 0), stop=(k == 15),
                )
        # --- BN stats over batch (free dim) ---
        for m in range(4):
            nc.vector.reduce_sum(mu_s[:, m : m + 1], gb[0][:, m * B : (m + 1) * B], axis=AX.X)
        nc.scalar.activation(tsq_s[:], gb[0][:, 0:128], ACT.Square)
        for m in range(4):
            nc.vector.reduce_sum(e2_s[:, m : m + 1], tsq_s[:, m * B : (m + 1) * B], axis=AX.X)
        nc.scalar.mul(mu_s[:], mu_s[:], 1.0 / B)
        nc.scalar.mul(e2_s[:], e2_s[:], 1.0 / B)
        nc.vector.tensor_mul(var_s[:], mu_s[:], mu_s[:])
        nc.vector.tensor_sub(var_s[:], e2_s[:], var_s[:])
        nc.vector.tensor_scalar_add(var_s[:], var_s[:], EPS)
        nc.scalar.activation(var_s[:], var_s[:], ACT.Sqrt)
        nc.vector.reciprocal(var_s[:], var_s[:])
        nc.vector.tensor_mul(sc_s[:], bng_s[:], var_s[:])
        nc.vector.tensor_mul(sh_s[:], mu_s[:], sc_s[:])
        nc.vector.tensor_sub(sh_s[:], bnb_s[:], sh_s[:])
        for m in range(4):
            nc.scalar.activation(
                xsT_s[:, m * 2048 : m * 2048 + B], gb[0][:, m * B : (m + 1) * B],
                ACT.Identity, bias=sh_s[:, m : m + 1], scale=sc_s[:, m : m + 1],
            )

        # --- bulk pre0 = Wih0 @ xs (token-major, bias folded) -> xp ---
        pre_banks = [fbk[0], fbk[1], gb[0], gb[1]]
        for c in range(4):
            for j in range(4):
                pb = pre_banks[j]
                for k in range(4):
                    nc.tensor.matmul(
                        pb[:, 0:512],
                        w0i_s[:, (j * 4 + k) * 128 : (j * 4 + k + 1) * 128],
                        xsT_s[:, k * 2048 + c * 512 : k * 2048 + (c + 1) * 512],
                        start=(k == 0), stop=(k == 3),
                    )
            for j in range(4):
                pb = pre_banks[j]
                dst = xp_s[:].rearrange("p (t jj b) -> p t jj b", t=64, jj=4)[
                    :, c * 16 : (c + 1) * 16, j, :
                ]
                nc.scalar.activation(dst, pb[:, 0:512], ACT.Identity, bias=b0_s[:, j : j + 1])

        nc.vector.memset(c01_s[:], 0.0)

        # --- FC work queue: (chunk, mtile) over 256-col (8-token) chunks,
        # ready once hs1 holds the chunk's tokens (token tt lands at stage
        # tt+3). Finer chunks start earlier and shrink the post-loop tail.
        fc_queue = [(c, m) for c in range(8) for m in range(32)]
        fc_idx = 0
        fc_count = 0

        def emit_fc(c, m):
            nonlocal fc_count
            pb = fbk[fc_count % 4]
            for k in range(8):
                nc.tensor.matmul(
                    pb[:, 0:256],
                    fcw_s[:, (m * 8 + k) * 128 : (m * 8 + k + 1) * 128],
                    hs1_s[:, k * 2048 + c * 256 : k * 2048 + (c + 1) * 256],
                    start=(k == 0), stop=(k == 7),
                )
            # bias-add on DVE; stores alternate between the two HWDGE rings
            # (4 staging buffers + 4 psum banks hide the ~2.7us store fixed
            # cost that was throttling the FC drain)
            st = fst_s[fc_count % 4]
            nc.vector.tensor_scalar_add(st[:], pb[:, 0:256], fcb_s[:, m : m + 1])
            rows = 128 if m < 31 else VS - 31 * 128
            ring = nc.sync.dma_start if fc_count % 2 else nc.scalar.dma_start
            ring(out=d_out[m * 128 : m * 128 + rows, c * 256 : (c + 1) * 256],
                 in_=st[0:rows, :])
            fc_count += 1

        # --- 66 pipeline stages: L0 step t, L1 step t-2, one AllGather ---
        # Both layers' gates share ONE psum bank per parity (L0 at cols
        # 0:128, L1 at 128:256) so a single merged act-chain + a single
        # send DMA serves both, halving the post-landing serial path.
        for t in range(T + 2):
            hb = hbuf_s[t % 4]              # landed: h0_{t-1} | h1_{t-3}
            ohb = hbuf_s[(t - 1) % 4]       # older: h0_{t-2} | h1_{t-4}
            snd = hsend_s[t % 2]
            g = gb[t % 2]
            sg = sgc_s[t % 2]
            l0a = t < T                     # L0 active
            l1a = 2 <= t                    # L1 active
            have_w0h = 1 <= t < T
            have_w1h = 3 <= t

            # landing of AG_{t-1}: split across the two HWDGE queues so the
            # two partition halves drain in parallel after the AG completes
            if t >= 1:
                full = cc_out[(t - 1) % 4][:].rearrange("(p w) c -> p (w c)", p=128)
                dma(out=hb[0:64, 0:512], in_=full[0:64])
                nc.scalar.dma_start(out=hb[64:128, 0:512], in_=full[64:128])

            started = False
            last_slack = None
            first_gated = None
            # slack phase (no dependency on this stage's landing):
            # L1 w1i on h0_{t-2}, the two bias/xp identity matmuls
            if l1a:
                for j in range(4):
                    for k in range(8):
                        last_slack = nc.tensor.matmul(
                            g[:, 128 + j * B : 128 + (j + 1) * B],
                            w1i_s[:, (j * 8 + k) * 128 : (j * 8 + k + 1) * 128],
                            ohb[:, k * 64 : k * 64 + B],
                            start=not started, stop=False,
                        )
                        started = True
                last_slack = nc.tensor.matmul(
                    g[:, 128:256], ident_s[:], b1bc_s[:], start=False, stop=False,
                )
            if l0a:
                last_slack = nc.tensor.matmul(
                    g[:, 0:128], ident_s[:], xp_s[:, t * 128 : (t + 1) * 128],
                    start=not started,
                    stop=not (have_w0h or have_w1h),
                )
                started = True

            # post-landing phase: the recurrence matmuls
            if have_w0h:
                for k in range(8):
                    for j in range(4):
                        mm = nc.tensor.matmul(
                            g[:, j * B : (j + 1) * B],
                            w0h_s[:, (j * 8 + k) * 128 : (j * 8 + k + 1) * 128],
                            hb[:, k * 64 : k * 64 + B],
                            start=False,
                            stop=(not have_w1h) and k == 7 and j == 3,
                        )
                        if first_gated is None:
                            first_gated = mm
            if have_w1h:
                for k in range(8):
                    for j in range(4):
                        mm = nc.tensor.matmul(
                            g[:, 128 + j * B : 128 + (j + 1) * B],
                            w1h_s[:, (j * 8 + k) * 128 : (j * 8 + k + 1) * 128],
                            hb[:, k * 64 + B : (k + 1) * 64],
                            start=False, stop=k == 7 and j == 3,
                        )
                        if first_gated is None:
                            first_gated = mm
            if first_gated is not None and last_slack is not None:
                # keep the landing-gated matmuls BEHIND the slack matmuls in
                # the PE FIFO so w1i/idents run inside the AllGather window
                add_dep_helper(first_gated.ins, last_slack.ins, False,
                               "slack mms before landing-gated mms")

            # merged act-chain over the active layer halves
            lo, hi = (0 if l0a else 1), (2 if l1a else 1)
            gv = g[:, 0:256].rearrange("p (l c) -> p l c", l=2)[:, lo:hi]
            sv = sg[:, 0:256].rearrange("p (l c) -> p l c", l=2)[:, lo:hi]
            cv = c01_s[:].rearrange("p (l c) -> p l c", l=2)[:, lo:hi]
            m1v = m1_s[:].rearrange("p (l c) -> p l c", l=2)[:, lo:hi]
            m2v = m2_s[:].rearrange("p (l c) -> p l c", l=2)[:, lo:hi]
            tcv = tc01_s[:].rearrange("p (l c) -> p l c", l=2)[:, lo:hi]
            sndv = snd[:, 0:64].rearrange("p (l c) -> p l c", l=2)[:, lo:hi]
            nc.scalar.activation(sv[:, :, 0:96], gv[:, :, 0:96], ACT.Sigmoid)
            nc.scalar.activation(sv[:, :, 96:128], gv[:, :, 96:128], ACT.Tanh)
            nc.vector.tensor_mul(m1v, sv[:, :, 32:64], cv)
            nc.vector.tensor_mul(m2v, sv[:, :, 0:32], sv[:, :, 96:128])
            nc.vector.tensor_add(cv, m1v, m2v)
            nc.scalar.activation(tcv, cv, ACT.Tanh)
            nc.vector.tensor_mul(sndv, sv[:, :, 64:96], tcv)
            dma(out=cc_in[t % 4][:, 0:64], in_=snd[:, 0:64])

            nc.gpsimd.collective_compute(
                "AllGather",
                mybir.AluOpType.bypass,
                replica_groups=[list(range(NC))],
                ins=[cc_in[t % 4][:]],
                outs=[cc_out[t % 4][:]],
            )

            # FC AFTER the send in every queue: the matmuls fill the
            # AllGather flight window on the PE, and the stores enqueue
            # behind (not ahead of) the critical send/landing transfers
            emitted = 0
            while fc_idx < len(fc_queue) and emitted < 5:
                c, m = fc_queue[fc_idx]
                if t < 8 * c + 11:
                    break
                emit_fc(c, m)
                fc_idx += 1
                emitted += 1

            # hs1 assembly: h1_{t-3} from this stage's landed buffer
            if t >= 3:
                src = hb[:].rearrange("p (s c) -> p s c", s=NC)[:, :, B:64]
                dst = hs1_s[:].rearrange("p (s tok) -> p s tok", s=NC)[
                    :, :, (t - 3) * B : (t - 2) * B
                ]
                nc.vector.tensor_copy(dst, src)

        # final harvest: h1_{T-1} was gathered by the stage-(T+1) AllGather
        fhb = hbuf_s[(T + 2) % 4]
        dma(
            out=fhb[:, 0:512],
            in_=cc_out[(T + 1) % 4][:].rearrange("(p w) c -> p (w c)", p=128),
        )
        src = fhb[:].rearrange("p (s c) -> p s c", s=NC)[:, :, B:64]
        dst = hs1_s[:].rearrange("p (s tok) -> p s tok", s=NC)[
            :, :, (T - 1) * B : T * B
        ]
        nc.vector.tensor_copy(dst, src)

        while fc_idx < len(fc_queue):
            c, m = fc_queue[fc_idx]
            emit_fc(c, m)
            fc_idx += 1

    nc.finalize()
    _nc_cache = nc
    return nc


def kernel(**inputs):
    inputs = {k: np.asarray(v) for k, v in inputs.items()}
    in_maps = _prep(inputs)
    nc = _build()
    res = run_bass_kernel_spmd(nc, in_maps, core_ids=list(range(NC)))
    big = np.concatenate([np.asarray(res.results[r]["out"]) for r in range(NC)], axis=0)
    return np.ascontiguousarray(
        big.reshape(V, T, B).transpose(2, 1, 0)
    ).astype(np.float32)



# revision 18
# speedup vs baseline: 1.0012x; 1.0012x over previous
"""Distributed Trainium2 kernel for the image-captioning model
(Linear+BN image embed -> 2-layer LSTM (T=64) -> H->V=32000 projection).

Sharding: the LSTM hidden state is sharded over the 4H gate dimension
(each of the 8 cores owns 128 h-positions of each layer); the per-step
full-h is re-assembled with one AllGather per pipeline stage (layer 1
runs one step behind layer 0, so both layers share a single exchange).
The fc projection is sharded over V (4000 rows/core) and interleaved
into the PE idle gaps of the recurrence. All matmuls run in bf16 with
fp32 PSUM accumulation.
"""
import numpy as np
import ml_dtypes

import concourse.bass as bass
import concourse.bacc as bacc
import concourse.mybir as mybir
from concourse.tile import TileContext
from concourse.tile_rust import add_dep_helper
from concourse.bass_utils import run_bass_kernel_spmd

BF16 = ml_dtypes.bfloat16
F32 = np.float32

V, H, E, B, T = 32000, 1024, 512, 32, 64
NC = 8
HS = H // NC          # 128 h-positions per core
VS = V // NC          # 4000 vocab rows per core
TOK = B * T           # 2048 tokens, col = t*32 + b
EPS = 1e-5
LAG = 4  # layer-1 pipeline lag (Wih1 batched per 4 steps)
# gate slot order in psum: i, f, o, g  (PyTorch weight row blocks i,f,g,o)
GATE_BASE = [0, 1, 3, 2]

_nc_cache = None


# The gathered h lands in SBUF via one contiguous 1KB-per-partition DMA
# (cc_out row 8p+j -> partition p, block j), so the K dim of every
# weight that consumes gathered h is pre-permuted to match: tile j,
# K-partition p draws from h index 8p+j.
_HPERM = (8 * np.arange(128)[None, :] + np.arange(8)[:, None]).reshape(-1)


def _gate_tiles(W, r, nk, interleave=False):
    """Per-core lhsT tile layout for a 4H-row weight: returns
    [128, 4*nk*128] with tile (j, k) at cols (j*nk+k)*128."""
    blocks = np.stack(
        [W[GATE_BASE[j] * H + r * HS : GATE_BASE[j] * H + r * HS + HS, :].T
         for j in range(4)]
    )  # (4, K, 128)
    K = blocks.shape[1]
    assert K == nk * 128
    if interleave:
        assert nk == 8
        blocks = blocks[:, _HPERM, :]
    return np.ascontiguousarray(
        blocks.reshape(4, nk, 128, 128).transpose(2, 0, 1, 3).reshape(128, 4 * nk * 128)
    ).astype(BF16)


def _prep(inputs):
    imgT = np.ascontiguousarray(inputs["image_feat"].T)  # (2048, 32)
    # rhs tiles [128, 16*32]
    imgT_s = np.ascontiguousarray(
        imgT.reshape(16, 128, B).transpose(1, 0, 2).reshape(128, 16 * B)
    ).astype(BF16)
    # lin lhsT tiles [128, 16k*4m*128]
    A = inputs["lin_W"].T  # (2048, 512)
    linWT = np.ascontiguousarray(
        A.reshape(16, 128, 4, 128).transpose(1, 0, 2, 3).reshape(128, 8192)
    ).astype(BF16)

    def col4(x):
        return np.ascontiguousarray(x.reshape(4, 128).T).astype(F32)

    bng = col4(inputs["bn_gamma"])
    bnb = col4(inputs["bn_beta"])

    caps = np.asarray(inputs["captions"])[:, : T - 1]  # (32, 63)
    cap_emb = inputs["emb"][caps]  # (32, 63, 512)
    capT = np.ascontiguousarray(cap_emb.transpose(2, 1, 0).reshape(E, (T - 1) * B)).astype(BF16)

    ident = np.eye(128, dtype=BF16)

    common = {
        "imgT": imgT_s, "linWT": linWT, "bng": bng, "bnb": bnb,
        "capT": capT, "ident": ident,
    }

    b0 = inputs["lstm_bih0"] + inputs["lstm_bhh0"]
    b1 = inputs["lstm_bih1"] + inputs["lstm_bhh1"]
    fcW = inputs["fc_W"]
    fcb_full = inputs["fc_b"]

    in_maps = []
    for r in range(NC):
        m = dict(common)
        m["w0i"] = _gate_tiles(inputs["lstm_Wih0"], r, 4)   # (128, 2048)
        m["w0h"] = _gate_tiles(inputs["lstm_Whh0"], r, 8, True)   # (128, 4096)
        m["w1i"] = _gate_tiles(inputs["lstm_Wih1"], r, 8, True)
        m["w1h"] = _gate_tiles(inputs["lstm_Whh1"], r, 8, True)
        m["b0"] = np.ascontiguousarray(
            np.stack([b0[GATE_BASE[j] * H + r * HS : GATE_BASE[j] * H + r * HS + HS]
                      for j in range(4)], axis=1)).astype(F32)  # (128, 4)
        m["b1"] = np.ascontiguousarray(
            np.stack([b1[GATE_BASE[j] * H + r * HS : GATE_BASE[j] * H + r * HS + HS]
                      for j in range(4)], axis=1)).astype(F32)
        m["b1bc"] = np.ascontiguousarray(np.repeat(m["b1"], B, axis=1)).astype(BF16)
        F = np.zeros((4096, H), np.float32)
        F[:VS] = fcW[r * VS : (r + 1) * VS]
        m["fcw"] = np.ascontiguousarray(
            F.T[_HPERM].reshape(8, 128, 32, 128).transpose(1, 2, 0, 3)
            .reshape(128, 32768)
        ).astype(BF16)
        fb = np.zeros((4096,), np.float32)
        fb[:VS] = fcb_full[r * VS : (r + 1) * VS]
        m["fcb"] = np.ascontiguousarray(fb.reshape(32, 128).T).astype(F32)
        in_maps.append(m)
    return in_maps


def _build():
    global _nc_cache
    if _nc_cache is not None:
        return _nc_cache
    bf = mybir.dt.bfloat16
    f32 = mybir.dt.float32
    nc = bacc.Bacc("TRN2", target_bir_lowering=False, debug=False)

    P = nc.declare_dram_parameter
    d_imgT = P("imgT", [128, 16 * B], bf, isOutput=False)
    d_linWT = P("linWT", [128, 8192], bf, isOutput=False)
    d_bng = P("bng", [128, 4], f32, isOutput=False)
    d_bnb = P("bnb", [128, 4], f32, isOutput=False)
    d_capT = P("capT", [E, (T - 1) * B], bf, isOutput=False)
    d_ident = P("ident", [128, 128], bf, isOutput=False)
    d_w0i = P("w0i", [128, 2048], bf, isOutput=False)
    d_w0h = P("w0h", [128, 4096], bf, isOutput=False)
    d_w1i = P("w1i", [128, 4096], bf, isOutput=False)
    d_w1h = P("w1h", [128, 4096], bf, isOutput=False)
    d_b0 = P("b0", [128, 4], f32, isOutput=False)
    d_b1 = P("b1", [128, 4], f32, isOutput=False)
    d_b1bc = P("b1bc", [128, 128], bf, isOutput=False)
    d_fcw = P("fcw", [128, 32768], bf, isOutput=False)
    d_fcb = P("fcb", [128, 32], f32, isOutput=False)
    d_out = P("out", [VS, TOK], f32, isOutput=True)

    S = nc.alloc_sbuf_tensor
    linWT_s = S("linWT_s", [128, 8192], bf)
    imgT_s = S("imgT_s", [128, 16 * B], bf)
    w0i_s = S("w0i_s", [128, 2048], bf)
    w0h_s = S("w0h_s", [128, 4096], bf)
    w1i_s = S("w1i_s", [128, 4096], bf)
    w1h_s = S("w1h_s", [128, 4096], bf)
    fcw_s = S("fcw_s", [128, 32768], bf)
    xsT_s = S("xsT_s", [128, 8192], bf)
    xp_s = S("xp_s", [128, 8192], bf)
    hs1_s = S("hs1_s", [128, 16384], bf)
    hbuf_s = [S(f"hbuf{i}", [128, NC * 64], bf) for i in range(4)]
    hsend_s = [S(f"hsend{i}", [128, 64], bf) for i in range(2)]
    fst_s = [S(f"fst{i}", [128, 256], f32) for i in range(4)]
    c01_s = S("c01_s", [128, 2 * B], f32)
    sgc_s = [S(f"sgc{i}", [128, 256], f32) for i in range(2)]
    tc01_s = S("tc01_s", [128, 2 * B], f32)
    m1_s = S("m1_s", [128, 2 * B], f32)
    m2_s = S("m2_s", [128, 2 * B], f32)
    b0_s = S("b0_s", [128, 4], f32)
    b1_s = S("b1_s", [128, 4], f32)
    b1bc_s = S("b1bc_s", [128, 128], bf)
    fcb_s = S("fcb_s", [128, 32], f32)
    bng_s = S("bng_s", [128, 4], f32)
    bnb_s = S("bnb_s", [128, 4], f32)
    ident_s = S("ident_s", [128, 128], bf)
    mu_s = S("mu_s", [128, 4], f32)
    e2_s = S("e2_s", [128, 4], f32)
    var_s = S("var_s", [128, 4], f32)
    sc_s = S("sc_s", [128, 4], f32)
    sh_s = S("sh_s", [128, 4], f32)
    tsq_s = S("tsq_s", [128, 128], f32)

    PS = nc.alloc_psum_tensor
    gb = [PS(f"gb{i}", [128, 512], f32) for i in range(2)]
    fbk = [PS(f"fb{i}", [128, 512], f32) for i in range(4)]

    warm_in = nc.dram_tensor("warm_in", [1, 16], f32)
    warm_out = nc.dram_tensor("warm_out", [1, 16], f32, addr_space="Shared")
    cc_in = [nc.dram_tensor(f"cc_in{i}", [128, 64], bf) for i in range(2)]
    cc_out = [nc.dram_tensor(f"cc_out{i}", [NC * 128, 64], bf, addr_space="Shared")
              for i in range(2)]

    ACT = mybir.ActivationFunctionType
    AX = mybir.AxisListType

    with TileContext(nc) as tc:
        dma = nc.sync.dma_start
        # warm up the collective path concurrently with the prologue
        nc.gpsimd.collective_compute(
            "AllReduce",
            mybir.AluOpType.add,
            replica_groups=[list(range(NC))],
            ins=[warm_in[:]],
            outs=[warm_out[:]],
        )
        # --- weight / const loads ---
        dma(out=linWT_s[:], in_=d_linWT[:])
        dma(out=imgT_s[:], in_=d_imgT[:])
        dma(out=w0i_s[:], in_=d_w0i[:])
        dma(out=w0h_s[:], in_=d_w0h[:])
        dma(out=w1i_s[:], in_=d_w1i[:])
        dma(out=w1h_s[:], in_=d_w1h[:])
        dma(out=b0_s[:], in_=d_b0[:])
        dma(out=b1_s[:], in_=d_b1[:])
        dma(out=b1bc_s[:], in_=d_b1bc[:])
        dma(out=fcb_s[:], in_=d_fcb[:])
        dma(out=bng_s[:], in_=d_bng[:])
        dma(out=bnb_s[:], in_=d_bnb[:])
        dma(out=ident_s[:], in_=d_ident[:])
        for k in range(4):
            nc.scalar.dma_start(out=xsT_s[:, k * 2048 + B : (k + 1) * 2048],
                                in_=d_capT[k * 128 : (k + 1) * 128, :])
        nc.scalar.dma_start(out=fcw_s[:], in_=d_fcw[:])

        # --- image embed: x.T tiles -> gb[0][:, 0:128] ---
        for m in range(4):
            for k in range(16):
                nc.tensor.matmul(
                    gb[0][:, m * B : (m + 1) * B],
                    linWT_s[:, (k * 4 + m) * 128 : (k * 4 + m + 1) * 128],
                    imgT_s[:, k * B : (k + 1) * B],
                    start=(k == 0), stop=(k == 15),
                )
        # --- BN stats over batch (free dim) ---
        for m in range(4):
            nc.vector.reduce_sum(mu_s[:, m : m + 1], gb[0][:, m * B : (m + 1) * B], axis=AX.X)
        nc.scalar.activation(tsq_s[:], gb[0][:, 0:128], ACT.Square)
        for m in range(4):
            nc.vector.reduce_sum(e2_s[:, m : m + 1], tsq_s[:, m * B : (m + 1) * B], axis=AX.X)
        nc.scalar.mul(mu_s[:], mu_s[:], 1.0 / B)
        nc.scalar.mul(e2_s[:], e2_s[:], 1.0 / B)
        nc.vector.tensor_mul(var_s[:], mu_s[:], mu_s[:])
        nc.vector.tensor_sub(var_s[:], e2_s[:], var_s[:])
        nc.vector.tensor_scalar_add(var_s[:], var_s[:], EPS)
        nc.scalar.activation(var_s[:], var_s[:], ACT.Sqrt)
        nc.vector.reciprocal(var_s[:], var_s[:])
        nc.vector.tensor_mul(sc_s[:], bng_s[:], var_s[:])
        nc.vector.tensor_mul(sh_s[:], mu_s[:], sc_s[:])
        nc.vector.tensor_sub(sh_s[:], bnb_s[:], sh_s[:])
        for m in range(4):
            nc.scalar.activation(
                xsT_s[:, m * 2048 : m * 2048 + B], gb[0][:, m * B : (m + 1) * B],
                ACT.Identity, bias=sh_s[:, m : m + 1], scale=sc_s[:, m : m + 1],
            )

        # --- bulk pre0 = Wih0 @ xs (token-major, bias folded) -> xp ---
        pre_banks = [fbk[0], fbk[1], gb[0], gb[1]]
        for c in range(4):
            for j in range(4):
                pb = pre_banks[j]
                for k in range(4):
                    nc.tensor.matmul(
                        pb[:, 0:512],
                        w0i_s[:, (j * 4 + k) * 128 : (j * 4 + k + 1) * 128],
                        xsT_s[:, k * 2048 + c * 512 : k * 2048 + (c + 1) * 512],
                        start=(k == 0), stop=(k == 3),
                    )
            for j in range(4):
                pb = pre_banks[j]
                dst = xp_s[:].rearrange("p (t jj b) -> p t jj b", t=64, jj=4)[
                    :, c * 16 : (c + 1) * 16, j, :
                ]
                nc.scalar.activation(dst, pb[:, 0:512], ACT.Identity, bias=b0_s[:, j : j + 1])

        nc.vector.memset(c01_s[:], 0.0)

        # --- FC work queue: (chunk, mtile) over 256-col (8-token) chunks,
        # ready once hs1 holds the chunk's tokens (token tt lands at stage
        # tt+3). Finer chunks start earlier and shrink the post-loop tail.
        fc_queue = [(c, m) for c in range(8) for m in range(32)]
        fc_idx = 0
        fc_count = 0

        def emit_fc(c, m):
            nonlocal fc_count
            pb = fbk[fc_count % 4]
            for k in range(8):
                nc.tensor.matmul(
                    pb[:, 0:256],
                    fcw_s[:, (m * 8 + k) * 128 : (m * 8 + k + 1) * 128],
                    hs1_s[:, k * 2048 + c * 256 : k * 2048 + (c + 1) * 256],
                    start=(k == 0), stop=(k == 7),
                )
            # bias-add on DVE; stores alternate between the two HWDGE rings
            # (4 staging buffers + 4 psum banks hide the ~2.7us store fixed
            # cost that was throttling the FC drain)
            st = fst_s[fc_count % 4]
            nc.vector.tensor_scalar_add(st[:], pb[:, 0:256], fcb_s[:, m : m + 1])
            rows = 128 if m < 31 else VS - 31 * 128
            ring = nc.sync.dma_start if fc_count % 2 else nc.scalar.dma_start
            ring(out=d_out[m * 128 : m * 128 + rows, c * 256 : (c + 1) * 256],
                 in_=st[0:rows, :])
            fc_count += 1

        # --- 66 pipeline stages: L0 step t, L1 step t-2, one AllGather ---
        # Both layers' gates share ONE psum bank per parity (L0 at cols
        # 0:128, L1 at 128:256) so a single merged act-chain + a single
        # send DMA serves both, halving the post-landing serial path.
        for t in range(T + 2):
            hb = hbuf_s[t % 4]              # landed: h0_{t-1} | h1_{t-3}
            ohb = hbuf_s[(t - 1) % 4]       # older: h0_{t-2} | h1_{t-4}
            snd = hsend_s[t % 2]
            g = gb[t % 2]
            sg = sgc_s[t % 2]
            l0a = t < T                     # L0 active
            l1a = 2 <= t                    # L1 active
            have_w0h = 1 <= t < T
            have_w1h = 3 <= t

            # landing of AG_{t-1}: split across the two HWDGE queues so the
            # two partition halves drain in parallel after the AG completes
            if t >= 1:
                full = cc_out[(t - 1) % 2][:].rearrange("(p w) c -> p (w c)", p=128)
                dma(out=hb[0:64, 0:512], in_=full[0:64])
                nc.scalar.dma_start(out=hb[64:128, 0:512], in_=full[64:128])

            started = False
            last_slack = None
            first_gated = None
            # slack phase (no dependency on this stage's landing):
            # L1 w1i on h0_{t-2}, the two bias/xp identity matmuls
            if l1a:
                for j in range(4):
                    for k in range(8):
                        last_slack = nc.tensor.matmul(
                            g[:, 128 + j * B : 128 + (j + 1) * B],
                            w1i_s[:, (j * 8 + k) * 128 : (j * 8 + k + 1) * 128],
                            ohb[:, k * 64 : k * 64 + B],
                            start=not started, stop=False,
                        )
                        started = True
                last_slack = nc.tensor.matmul(
                    g[:, 128:256], ident_s[:], b1bc_s[:], start=False, stop=False,
                )
            if l0a:
                last_slack = nc.tensor.matmul(
                    g[:, 0:128], ident_s[:], xp_s[:, t * 128 : (t + 1) * 128],
                    start=not started,
                    stop=not (have_w0h or have_w1h),
                )
                started = True

            # post-landing phase: the recurrence matmuls
            if have_w0h:
                for k in range(8):
                    for j in range(4):
                        mm = nc.tensor.matmul(
                            g[:, j * B : (j + 1) * B],
                            w0h_s[:, (j * 8 + k) * 128 : (j * 8 + k + 1) * 128],
                            hb[:, k * 64 : k * 64 + B],
                            start=False,
                            stop=(not have_w1h) and k == 7 and j == 3,
                        )
                        if first_gated is None:
                            first_gated = mm
            if have_w1h:
                for k in range(8):
                    for j in range(4):
                        mm = nc.tensor.matmul(
                            g[:, 128 + j * B : 128 + (j + 1) * B],
                            w1h_s[:, (j * 8 + k) * 128 : (j * 8 + k + 1) * 128],
                            hb[:, k * 64 + B : (k + 1) * 64],
                            start=False, stop=k == 7 and j == 3,
                        )
                        if first_gated is None:
                            first_gated = mm
            if first_gated is not None and last_slack is not None:
                # keep the landing-gated matmuls BEHIND the slack matmuls in
                # the PE FIFO so w1i/idents run inside the AllGather window
                add_dep_helper(first_gated.ins, last_slack.ins, False,
                               "slack mms before landing-gated mms")

            # merged act-chain over the active layer halves
            lo, hi = (0 if l0a else 1), (2 if l1a else 1)
            gv = g[:, 0:256].rearrange("p (l c) -> p l c", l=2)[:, lo:hi]
            sv = sg[:, 0:256].rearrange("p (l c) -> p l c", l=2)[:, lo:hi]
            cv = c01_s[:].rearrange("p (l c) -> p l c", l=2)[:, lo:hi]
            m1v = m1_s[:].rearrange("p (l c) -> p l c", l=2)[:, lo:hi]
            m2v = m2_s[:].rearrange("p (l c) -> p l c", l=2)[:, lo:hi]
            tcv = tc01_s[:].rearrange("p (l c) -> p l c", l=2)[:, lo:hi]
            sndv = snd[:, 0:64].rearrange("p (l c) -> p l c", l=2)[:, lo:hi]
            nc.scalar.activation(sv[:, :, 0:96], gv[:, :, 0:96], ACT.Sigmoid)
            nc.scalar.activation(sv[:, :, 96:128], gv[:, :, 96:128], ACT.Tanh)
            nc.vector.tensor_mul(m1v, sv[:, :, 32:64], cv)
            nc.vector.tensor_mul(m2v, sv[:, :, 0:32], sv[:, :, 96:128])
            nc.vector.tensor_add(cv, m1v, m2v)
            nc.scalar.activation(tcv, cv, ACT.Tanh)
            nc.vector.tensor_mul(sndv, sv[:, :, 64:96], tcv)
            dma(out=cc_in[t % 2][:, 0:64], in_=snd[:, 0:64])

            nc.gpsimd.collective_compute(
                "AllGather",
                mybir.AluOpType.bypass,
                replica_groups=[list(range(NC))],
                ins=[cc_in[t % 2][:]],
                outs=[cc_out[t % 2][:]],
            )

            # FC AFTER the send in every queue: the matmuls fill the
            # AllGather flight window on the PE, and the stores enqueue
            # behind (not ahead of) the critical send/landing transfers
            emitted = 0
            while fc_idx < len(fc_queue) and emitted < 5:
                c, m = fc_queue[fc_idx]
                if t < 8 * c + 11:
                    break
                emit_fc(c, m)
                fc_idx += 1
                emitted += 1

            # hs1 assembly: h1_{t-3} from this stage's landed buffer
            if t >= 3:
                src = hb[:].rearrange("p (s c) -> p s c", s=NC)[:, :, B:64]
                dst = hs1_s[:].rearrange("p (s tok) -> p s tok", s=NC)[
                    :, :, (t - 3) * B : (t - 2) * B
                ]
                nc.vector.tensor_copy(dst, src)

        # final harvest: h1_{T-1} was gathered by the stage-(T+1) AllGather
        fhb = hbuf_s[(T + 2) % 4]
        dma(
            out=fhb[:, 0:512],
            in_=cc_out[(T + 1) % 2][:].rearrange("(p w) c -> p (w c)", p=128),
        )
        src = fhb[:].rearrange("p (s c) -> p s c", s=NC)[:, :, B:64]
        dst = hs1_s[:].rearrange("p (s tok) -> p s tok", s=NC)[
            :, :, (T - 1) * B : T * B
        ]
        nc.vector.tensor_copy(dst, src)

        while fc_idx < len(fc_queue):
            c, m = fc_queue[fc_idx]
            emit_fc(c, m)
            fc_idx += 1

    nc.finalize()
    _nc_cache = nc
    return nc


def kernel(**inputs):
    inputs = {k: np.asarray(v) for k, v in inputs.items()}
    in_maps = _prep(inputs)
    nc = _build()
    res = run_bass_kernel_spmd(nc, in_maps, core_ids=list(range(NC)))
    big = np.concatenate([np.asarray(res.results[r]["out"]) for r in range(NC)], axis=0)
    return np.ascontiguousarray(
        big.reshape(V, T, B).transpose(2, 1, 0)
    ).astype(np.float32)

